# revision 14
# baseline (speedup 1.0000x reference)
"""Trainium2 Bass kernel v7 for nn_GeneralNetworkedAE (gnn_message_passing).

Measured subsystem floors on current hw (ablate.py, repeat-delta):
  DMA  (fp8 gT + bf16 out, 12.1 MB/iter):        26.8 us
  PE   (256 mm1 + 256 mm2 matmuls + LDW):        55.7 us
  DVE evac 1244 ns / ACT evac 1038 ns per FD=1024 PSUM->SBUF instr
  pe_evac (full compute structure, no DMA):      70.8 us   <- the wall

The evacuation is the hard wall: PSUM reads are 1 elem/cycle/lane, f32
only on TRN2, and only DVE+ACT have PSUM ports.  Concurrent DMA adds a
further ~10-20% slowdown to the compute engines (pure resource
contention, not dependencies — measured via pe_evac_gx).  A/B tests
(interleaved, same hw state — absolute timings drift +-15% between
runs): out-DMAs on gpsimd/SWDGE beat sync/HWDGE; gT loads must stay on
sync (SWDGE loads: catastrophic); FD=2048 evacs are blocked by the
8-bank PSUM budget (4-bank tiles x2 bufs leave no room for mm2, and
hpsum bufs=2 + 2-bank mm2 tiles measured 1.49x WORSE).

Final config: gT in fp8 e3m4 (half DMA traffic, end-to-end rel err
~1.5e-2 < 2e-2 with bf16 W1 — mixed-dtype matmul), outT unpadded
[896, BS] bf16, mm2 software-pipelined one group behind mm1, DVE/ACT
evacuation split by measured per-instr costs.  Measured 89-95 us
across hw states (baseline v4: 92-94 us in the same window).
"""

import numpy as np
import ml_dtypes

import concourse.bacc as bacc
import concourse.tile as tile
from concourse import mybir
from concourse.bass_utils import run_bass_kernel_spmd

BF16 = ml_dtypes.bfloat16
FP8 = ml_dtypes.float8_e3m4

B, NX, NU = 16384, 896, 128
A, DIN, H, DOUT = 32, 128, 256, 28
DOUTP = 32
N_CORES = 8
BS = B // N_CORES     # 2048 batch rows per core
BT = 512              # matmul moving free dim / psum bank
NT = BS // BT         # 4 batch tiles
NG = A // 4           # 8 groups of 4 agents

F32 = mybir.dt.float32
BF = mybir.dt.bfloat16
F8 = mybir.dt.float8e3


class EvacBalancer:
    def __init__(self):
        self.dve_ns = 0.0
        self.act_ns = 0.0

    def pick(self, fd):
        dve_cost = (120 + fd) / 0.96
        act_cost = (172 + fd) / 1.2
        if self.dve_ns + dve_cost <= self.act_ns + act_cost:
            self.dve_ns += dve_cost
            return "dve"
        self.act_ns += act_cost
        return "act"


def build_program(repeat: int = 1, out_eng: str = "gpsimd",
                  mm2_half: int = 0, gpool_bufs: int = 3,
                  hpsum_bufs: int = 3, flat_g: int = 1, g_split: int = 1):
    nc = bacc.Bacc(trn_type="TRN2", target_bir_lowering=False, debug=False,
                   enable_asserts=True)
    if flat_g:
        gT = nc.dram_tensor("gT", [DIN, A * BS], F8, kind="ExternalInput").ap()
    else:
        gT = nc.dram_tensor("gT", [A, DIN, BS], F8, kind="ExternalInput").ap()
    w1 = nc.dram_tensor("w1", [DIN, A * H], BF, kind="ExternalInput").ap()
    w2 = nc.dram_tensor("w2", [128, A * 2 * DOUTP], BF, kind="ExternalInput").ap()
    b1t = nc.dram_tensor("b1t", [128, A * 2], F32, kind="ExternalInput").ap()
    b2t = nc.dram_tensor("b2t", [128, NG], F32, kind="ExternalInput").ap()
    outT = nc.dram_tensor("outT", [A * DOUT, BS], BF, kind="ExternalOutput").ap()

    add = mybir.AluOpType.add
    mx = mybir.AluOpType.max
    relu = mybir.ActivationFunctionType.Relu
    ident = mybir.ActivationFunctionType.Identity

    bal = EvacBalancer()

    out_dma = nc.gpsimd.dma_start if out_eng == "gpsimd" else nc.sync.dma_start
    with tile.TileContext(nc) as tc:
        with (
            tc.tile_pool(name="wpool", bufs=1) as wpool,
            tc.tile_pool(name="gpool", bufs=gpool_bufs) as gpool,
            tc.tile_pool(name="hpool", bufs=18) as hpool,
            tc.tile_pool(name="opool", bufs=2) as opool,
            tc.tile_pool(name="hpsum", bufs=(2 if mm2_half else hpsum_bufs),
                         space="PSUM") as hpsum,
            tc.tile_pool(name="opsum", bufs=2, space="PSUM") as opsum,
        ):
            w1_head = wpool.tile([DIN, 4 * H], BF)
            nc.sync.dma_start(out=w1_head[:], in_=w1[:, :4 * H])
            w1_tail = wpool.tile([DIN, (A - 4) * H], BF)
            nc.gpsimd.dma_start(out=w1_tail[:], in_=w1[:, 4 * H:])

            def w1_slice(a, m):
                if a < 4:
                    return w1_head[:, a * H + m * 128:a * H + (m + 1) * 128]
                b = a - 4
                return w1_tail[:, b * H + m * 128:b * H + (m + 1) * 128]
            w2_sb = wpool.tile([128, A * 2 * DOUTP], BF)
            nc.gpsimd.dma_start(out=w2_sb[:], in_=w2[:])
            b1_sb = wpool.tile([128, A * 2], F32)
            nc.gpsimd.dma_start(out=b1_sb[:], in_=b1t[:])
            b2_sb = wpool.tile([128, NG], F32)
            nc.gpsimd.dma_start(out=b2_sb[:], in_=b2t[:])

            def evac(out_ap, in_ap, bcol, do_relu, fd):
                if bal.pick(fd) == "dve":
                    if do_relu:
                        nc.vector.tensor_scalar(
                            out=out_ap, in0=in_ap,
                            scalar1=bcol, scalar2=0.0, op0=add, op1=mx)
                    else:
                        nc.vector.tensor_scalar(
                            out=out_ap, in0=in_ap,
                            scalar1=bcol, scalar2=None, op0=add)
                else:
                    nc.scalar.activation(
                        out=out_ap, in_=in_ap,
                        func=(relu if do_relu else ident),
                        bias=bcol, scale=1.0)

            def emit_mm2_chunk(p, t):
                """One batch-tile of mm2 for a finished group p."""
                pg = p["g"]
                if mm2_half:
                    if t % 2 == 0:
                        ps_o2 = opsum.tile([128, 2 * BT], F32, tag="po")
                        p["ps_o"] = ps_o2
                    ps_o = p["ps_o"][:, (t % 2) * BT:(t % 2 + 1) * BT]
                else:
                    ps_o1 = opsum.tile([128, BT], F32, tag="po")
                    ps_o = ps_o1[:, :]
                for m in range(2):
                    for j in range(4):
                        a = 4 * pg + j
                        nc.tensor.matmul(
                            ps_o[32 * j:32 * j + DOUTP, :],
                            lhsT=w2_sb[:, (a * 2 + m) * DOUTP:
                                       (a * 2 + m + 1) * DOUTP],
                            rhs=p["hts"][(j, m)][:, t * BT:(t + 1) * BT],
                            start=(m == 0), stop=(m == 1),
                            tile_position=(0, 32 * j),
                            skip_group_check=True,
                        )
                bcol = b2_sb[:, pg:pg + 1]
                if mm2_half:
                    if t % 2 == 1:
                        evac(p["ostage"][:, (t - 1) * BT:(t + 1) * BT],
                             p["ps_o"][:], bcol, False, 2 * BT)
                else:
                    evac(p["ostage"][:, t * BT:(t + 1) * BT],
                         ps_o, bcol, False, BT)
                if t == NT - 1:
                    for j in range(4):
                        out_dma(
                            out=outT[pg * 4 * DOUT + j * DOUT:
                                     pg * 4 * DOUT + (j + 1) * DOUT, :],
                            in_=p["ostage"][32 * j:32 * j + DOUT, :])

            pending = None   # group whose mm2 lags one group behind
            for _r in range(repeat):
                for g in range(NG):
                    def g_src(a):
                        if flat_g:
                            return gT[:, a * BS:(a + 1) * BS]
                        return gT[a]

                    if g == 0 and _r == 0:
                        gts = []
                        for j in range(4):
                            g1 = wpool.tile([DIN, BS], F8, tag=f"g0a{j}")
                            nc.sync.dma_start(out=g1[:], in_=g_src(j))
                            gts.append(g1[:, :])
                    else:
                        gt4 = gpool.tile([DIN, 4 * BS], F8, tag="gt")
                        if g_split == 4:
                            for j in range(4):
                                nc.sync.dma_start(
                                    out=gt4[:, j * BS:(j + 1) * BS],
                                    in_=g_src(4 * g + j))
                        elif flat_g:
                            nc.sync.dma_start(
                                out=gt4[:],
                                in_=gT[:, 4 * g * BS:(4 * g + 4) * BS])
                        else:
                            nc.sync.dma_start(
                                out=gt4[:].rearrange("p (k c) -> p k c", k=4),
                                in_=gT[4 * g:4 * g + 4].rearrange(
                                    "k p c -> p k c"))
                        gts = [gt4[:, j * BS:(j + 1) * BS] for j in range(4)]
                    ostage = opool.tile([128, BS], BF, tag="ostage")

                    # ---- mm1 for group g, interleaved with mm2(g-1) ----
                    hts = {}
                    slot = 0
                    for j in range(4):
                        a = 4 * g + j
                        for m in range(2):
                            h_sb = hpool.tile([128, BS], BF, tag="h")
                            bcol = b1_sb[:, a * 2 + m:a * 2 + m + 1]
                            for half in range(2):          # t pairs
                                ps_h = hpsum.tile([128, 2 * BT], F32, tag="ph")
                                for tt in range(2):
                                    t = 2 * half + tt
                                    nc.tensor.matmul(
                                        ps_h[:, tt * BT:(tt + 1) * BT],
                                        lhsT=w1_slice(a, m),
                                        rhs=gts[j][:, t * BT:(t + 1) * BT],
                                        start=True, stop=True,
                                    )
                                evac(h_sb[:, half * 2 * BT:(half + 1) * 2 * BT],
                                     ps_h[:], bcol, True, 2 * BT)
                            hts[(j, m)] = h_sb
                            slot += 1
                            if pending is not None and slot % 2 == 0:
                                emit_mm2_chunk(pending, slot // 2 - 1)
                    pending = {"g": g, "hts": hts, "ostage": ostage}
            for t in range(NT):       # flush final group's mm2
                emit_mm2_chunk(pending, t)
            pending = None
    nc.compile()
    return nc


def prep_inputs(x, u, W1, b1, W2, b2, in_idx):
    """Host-side shard + layout prep. Returns per-core in_maps."""
    feats = np.concatenate([np.asarray(x, np.float32),
                            np.asarray(u, np.float32)], axis=1)  # [B, 1024]
    featsT = np.ascontiguousarray(feats.T).astype(FP8)           # [1024, B]
    flat_idx = np.asarray(in_idx).reshape(-1).astype(np.int64)
    gT_full = featsT[flat_idx]                                    # [A*DIN, B]

    w1h = np.asarray(W1, np.float32).transpose(1, 0, 2).reshape(DIN, A * H)
    w1h = np.ascontiguousarray(w1h).astype(BF16)
    w2p = np.zeros((A, H, DOUTP), np.float32)
    w2p[:, :, :DOUT] = np.asarray(W2, np.float32)
    w2h = (w2p.reshape(A, 2, 128, DOUTP).transpose(2, 0, 1, 3)
           .reshape(128, A * 2 * DOUTP))
    w2h = np.ascontiguousarray(w2h).astype(BF16)
    b1h = np.ascontiguousarray(
        np.asarray(b1, np.float32).reshape(A, 2, 128).transpose(2, 0, 1)
        .reshape(128, A * 2))
    b2h = np.zeros((128, NG), np.float32)
    for g in range(NG):
        for j in range(4):
            b2h[32 * j:32 * j + DOUT, g] = np.asarray(b2, np.float32)[4 * g + j]

    in_maps = []
    for c in range(N_CORES):
        gT_c = gT_full[:, c * BS:(c + 1) * BS].reshape(A, DIN, BS)
        # flat layout [DIN, A*BS]: agent-major within each partition row,
        # so a 4-agent group load is one 8KB-contiguous run per partition.
        gT_c = np.ascontiguousarray(
            gT_c.transpose(1, 0, 2).reshape(DIN, A * BS))
        in_maps.append({"gT": gT_c, "w1": w1h, "w2": w2h,
                        "b1t": b1h, "b2t": b2h})
    return in_maps


def assemble_output(results, x, u, out_idx):
    """Gather per-core oT outputs, un-transpose, apply out_idx scatter."""
    o_rows = np.concatenate(
        [np.asarray(results[c]["outT"], dtype=np.float32)
         for c in range(N_CORES)], axis=1)                # [A*DOUT, B]
    o_flat = np.ascontiguousarray(o_rows.T)               # [B, 896]
    oi = np.asarray(out_idx).reshape(-1).astype(np.int64)
    if np.array_equal(oi, np.arange(A * DOUT)):
        return o_flat
    feats = np.concatenate([np.asarray(x, np.float32),
                            np.asarray(u, np.float32)], axis=1)
    feats[:, oi] = o_flat
    return np.ascontiguousarray(feats[:, :NX])


def kernel(x, u, W1, b1, W2, b2, in_idx, out_idx):
    nc = build_program(repeat=1)
    in_maps = prep_inputs(x, u, W1, b1, W2, b2, in_idx)
    res = run_bass_kernel_spmd(nc, in_maps, core_ids=list(range(N_CORES)))
    return assemble_output(res.results, x, u, out_idx)


# revision 15
# speedup vs baseline: 1.0075x; 1.0075x over previous
"""Trainium2 Bass kernel v7 for nn_GeneralNetworkedAE (gnn_message_passing).

Measured subsystem floors on current hw (ablate.py, repeat-delta):
  DMA  (fp8 gT + bf16 out, 12.1 MB/iter):        26.8 us
  PE   (256 mm1 + 256 mm2 matmuls + LDW):        55.7 us
  DVE evac 1244 ns / ACT evac 1038 ns per FD=1024 PSUM->SBUF instr
  pe_evac (full compute structure, no DMA):      70.8 us   <- the wall

The evacuation is the hard wall: PSUM reads are 1 elem/cycle/lane, f32
only on TRN2, and only DVE+ACT have PSUM ports.  Concurrent DMA adds a
further ~10-20% slowdown to the compute engines (pure resource
contention, not dependencies — measured via pe_evac_gx).  A/B tests
(interleaved, same hw state — absolute timings drift +-15% between
runs): out-DMAs on gpsimd/SWDGE beat sync/HWDGE; gT loads must stay on
sync (SWDGE loads: catastrophic); FD=2048 evacs are blocked by the
8-bank PSUM budget (4-bank tiles x2 bufs leave no room for mm2, and
hpsum bufs=2 + 2-bank mm2 tiles measured 1.49x WORSE).

Interleaved A/B results (drift-robust, med of per-round ratios):
  flat gT layout [DIN, A*BS] (8KB contiguous runs, 1024 descs/iter
  vs 4096):                               0.974x  -> ADOPTED (default)
  per-agent load granularity (g_split=4): 1.122x  -> rejected
  gpool_bufs 4 vs 3:                      wash    -> keep 3
  hpsum needs 3 bufs so DVE and ACT can drain two tiles concurrently
  while PE fills a third (2 bufs measured 1.49x worse).

Final config: gT in fp8 e3m4, flat [DIN, A*BS] layout (half DMA
traffic; end-to-end rel err ~1.5e-2 < 2e-2 with bf16 W1 — mixed-dtype
matmul), outT unpadded [896, BS] bf16 via gpsimd/SWDGE, mm2
software-pipelined one group behind mm1, DVE/ACT evacuation split by
measured per-instr costs.  Measured 89-96 us across hw states
(baseline v4: 92-94 us in the same windows).
"""

import numpy as np
import ml_dtypes

import concourse.bacc as bacc
import concourse.tile as tile
from concourse import mybir
from concourse.bass_utils import run_bass_kernel_spmd

BF16 = ml_dtypes.bfloat16
FP8 = ml_dtypes.float8_e3m4

B, NX, NU = 16384, 896, 128
A, DIN, H, DOUT = 32, 128, 256, 28
DOUTP = 32
N_CORES = 8
BS = B // N_CORES     # 2048 batch rows per core
BT = 512              # matmul moving free dim / psum bank
NT = BS // BT         # 4 batch tiles
NG = A // 4           # 8 groups of 4 agents

F32 = mybir.dt.float32
BF = mybir.dt.bfloat16
F8 = mybir.dt.float8e3


class EvacBalancer:
    def __init__(self):
        self.dve_ns = 0.0
        self.act_ns = 0.0

    def pick(self, fd):
        dve_cost = (120 + fd) / 0.96
        act_cost = (172 + fd) / 1.2
        if self.dve_ns + dve_cost <= self.act_ns + act_cost:
            self.dve_ns += dve_cost
            return "dve"
        self.act_ns += act_cost
        return "act"


def build_program(repeat: int = 1, out_eng: str = "gpsimd",
                  mm2_half: int = 0, gpool_bufs: int = 3,
                  hpsum_bufs: int = 3, flat_g: int = 1, g_split: int = 1):
    nc = bacc.Bacc(trn_type="TRN2", target_bir_lowering=False, debug=False,
                   enable_asserts=True)
    if flat_g:
        gT = nc.dram_tensor("gT", [DIN, A * BS], F8, kind="ExternalInput").ap()
    else:
        gT = nc.dram_tensor("gT", [A, DIN, BS], F8, kind="ExternalInput").ap()
    w1 = nc.dram_tensor("w1", [DIN, A * H], BF, kind="ExternalInput").ap()
    w2 = nc.dram_tensor("w2", [128, A * 2 * DOUTP], BF, kind="ExternalInput").ap()
    b1t = nc.dram_tensor("b1t", [128, A * 2], F32, kind="ExternalInput").ap()
    b2t = nc.dram_tensor("b2t", [128, NG], F32, kind="ExternalInput").ap()
    outT = nc.dram_tensor("outT", [A * DOUT, BS], BF, kind="ExternalOutput").ap()

    add = mybir.AluOpType.add
    mx = mybir.AluOpType.max
    relu = mybir.ActivationFunctionType.Relu
    ident = mybir.ActivationFunctionType.Identity

    bal = EvacBalancer()

    out_dma = nc.gpsimd.dma_start if out_eng == "gpsimd" else nc.sync.dma_start
    with tile.TileContext(nc) as tc:
        with (
            tc.tile_pool(name="wpool", bufs=1) as wpool,
            tc.tile_pool(name="gpool", bufs=gpool_bufs) as gpool,
            tc.tile_pool(name="hpool", bufs=18) as hpool,
            tc.tile_pool(name="opool", bufs=2) as opool,
            tc.tile_pool(name="hpsum", bufs=(2 if mm2_half else hpsum_bufs),
                         space="PSUM") as hpsum,
            tc.tile_pool(name="opsum", bufs=2, space="PSUM") as opsum,
        ):
            w1_head = wpool.tile([DIN, 4 * H], BF)
            nc.sync.dma_start(out=w1_head[:], in_=w1[:, :4 * H])
            w1_tail = wpool.tile([DIN, (A - 4) * H], BF)
            nc.gpsimd.dma_start(out=w1_tail[:], in_=w1[:, 4 * H:])

            def w1_slice(a, m):
                if a < 4:
                    return w1_head[:, a * H + m * 128:a * H + (m + 1) * 128]
                b = a - 4
                return w1_tail[:, b * H + m * 128:b * H + (m + 1) * 128]
            w2_sb = wpool.tile([128, A * 2 * DOUTP], BF)
            nc.gpsimd.dma_start(out=w2_sb[:], in_=w2[:])
            b1_sb = wpool.tile([128, A * 2], F32)
            nc.gpsimd.dma_start(out=b1_sb[:], in_=b1t[:])
            b2_sb = wpool.tile([128, NG], F32)
            nc.gpsimd.dma_start(out=b2_sb[:], in_=b2t[:])

            def evac(out_ap, in_ap, bcol, do_relu, fd):
                if bal.pick(fd) == "dve":
                    if do_relu:
                        nc.vector.tensor_scalar(
                            out=out_ap, in0=in_ap,
                            scalar1=bcol, scalar2=0.0, op0=add, op1=mx)
                    else:
                        nc.vector.tensor_scalar(
                            out=out_ap, in0=in_ap,
                            scalar1=bcol, scalar2=None, op0=add)
                else:
                    nc.scalar.activation(
                        out=out_ap, in_=in_ap,
                        func=(relu if do_relu else ident),
                        bias=bcol, scale=1.0)

            def emit_mm2_chunk(p, t):
                """One batch-tile of mm2 for a finished group p."""
                pg = p["g"]
                if mm2_half:
                    if t % 2 == 0:
                        ps_o2 = opsum.tile([128, 2 * BT], F32, tag="po")
                        p["ps_o"] = ps_o2
                    ps_o = p["ps_o"][:, (t % 2) * BT:(t % 2 + 1) * BT]
                else:
                    ps_o1 = opsum.tile([128, BT], F32, tag="po")
                    ps_o = ps_o1[:, :]
                for m in range(2):
                    for j in range(4):
                        a = 4 * pg + j
                        nc.tensor.matmul(
                            ps_o[32 * j:32 * j + DOUTP, :],
                            lhsT=w2_sb[:, (a * 2 + m) * DOUTP:
                                       (a * 2 + m + 1) * DOUTP],
                            rhs=p["hts"][(j, m)][:, t * BT:(t + 1) * BT],
                            start=(m == 0), stop=(m == 1),
                            tile_position=(0, 32 * j),
                            skip_group_check=True,
                        )
                bcol = b2_sb[:, pg:pg + 1]
                if mm2_half:
                    if t % 2 == 1:
                        evac(p["ostage"][:, (t - 1) * BT:(t + 1) * BT],
                             p["ps_o"][:], bcol, False, 2 * BT)
                else:
                    evac(p["ostage"][:, t * BT:(t + 1) * BT],
                         ps_o, bcol, False, BT)
                if t == NT - 1:
                    for j in range(4):
                        out_dma(
                            out=outT[pg * 4 * DOUT + j * DOUT:
                                     pg * 4 * DOUT + (j + 1) * DOUT, :],
                            in_=p["ostage"][32 * j:32 * j + DOUT, :])

            pending = None   # group whose mm2 lags one group behind
            for _r in range(repeat):
                for g in range(NG):
                    def g_src(a):
                        if flat_g:
                            return gT[:, a * BS:(a + 1) * BS]
                        return gT[a]

                    if g == 0 and _r == 0:
                        gts = []
                        for j in range(4):
                            g1 = wpool.tile([DIN, BS], F8, tag=f"g0a{j}")
                            nc.sync.dma_start(out=g1[:], in_=g_src(j))
                            gts.append(g1[:, :])
                    else:
                        gt4 = gpool.tile([DIN, 4 * BS], F8, tag="gt")
                        if g_split == 4:
                            for j in range(4):
                                nc.sync.dma_start(
                                    out=gt4[:, j * BS:(j + 1) * BS],
                                    in_=g_src(4 * g + j))
                        elif flat_g:
                            nc.sync.dma_start(
                                out=gt4[:],
                                in_=gT[:, 4 * g * BS:(4 * g + 4) * BS])
                        else:
                            nc.sync.dma_start(
                                out=gt4[:].rearrange("p (k c) -> p k c", k=4),
                                in_=gT[4 * g:4 * g + 4].rearrange(
                                    "k p c -> p k c"))
                        gts = [gt4[:, j * BS:(j + 1) * BS] for j in range(4)]
                    ostage = opool.tile([128, BS], BF, tag="ostage")

                    # ---- mm1 for group g, interleaved with mm2(g-1) ----
                    hts = {}
                    slot = 0
                    for j in range(4):
                        a = 4 * g + j
                        for m in range(2):
                            h_sb = hpool.tile([128, BS], BF, tag="h")
                            bcol = b1_sb[:, a * 2 + m:a * 2 + m + 1]
                            for half in range(2):          # t pairs
                                ps_h = hpsum.tile([128, 2 * BT], F32, tag="ph")
                                for tt in range(2):
                                    t = 2 * half + tt
                                    nc.tensor.matmul(
                                        ps_h[:, tt * BT:(tt + 1) * BT],
                                        lhsT=w1_slice(a, m),
                                        rhs=gts[j][:, t * BT:(t + 1) * BT],
                                        start=True, stop=True,
                                    )
                                evac(h_sb[:, half * 2 * BT:(half + 1) * 2 * BT],
                                     ps_h[:], bcol, True, 2 * BT)
                            hts[(j, m)] = h_sb
                            slot += 1
                            if pending is not None and slot % 2 == 0:
                                emit_mm2_chunk(pending, slot // 2 - 1)
                    pending = {"g": g, "hts": hts, "ostage": ostage}
            for t in range(NT):       # flush final group's mm2
                emit_mm2_chunk(pending, t)
            pending = None
    nc.compile()
    return nc


def prep_inputs(x, u, W1, b1, W2, b2, in_idx):
    """Host-side shard + layout prep. Returns per-core in_maps."""
    feats = np.concatenate([np.asarray(x, np.float32),
                            np.asarray(u, np.float32)], axis=1)  # [B, 1024]
    featsT = np.ascontiguousarray(feats.T).astype(FP8)           # [1024, B]
    flat_idx = np.asarray(in_idx).reshape(-1).astype(np.int64)
    gT_full = featsT[flat_idx]                                    # [A*DIN, B]

    w1h = np.asarray(W1, np.float32).transpose(1, 0, 2).reshape(DIN, A * H)
    w1h = np.ascontiguousarray(w1h).astype(BF16)
    w2p = np.zeros((A, H, DOUTP), np.float32)
    w2p[:, :, :DOUT] = np.asarray(W2, np.float32)
    w2h = (w2p.reshape(A, 2, 128, DOUTP).transpose(2, 0, 1, 3)
           .reshape(128, A * 2 * DOUTP))
    w2h = np.ascontiguousarray(w2h).astype(BF16)
    b1h = np.ascontiguousarray(
        np.asarray(b1, np.float32).reshape(A, 2, 128).transpose(2, 0, 1)
        .reshape(128, A * 2))
    b2h = np.zeros((128, NG), np.float32)
    for g in range(NG):
        for j in range(4):
            b2h[32 * j:32 * j + DOUT, g] = np.asarray(b2, np.float32)[4 * g + j]

    in_maps = []
    for c in range(N_CORES):
        gT_c = gT_full[:, c * BS:(c + 1) * BS].reshape(A, DIN, BS)
        # flat layout [DIN, A*BS]: agent-major within each partition row,
        # so a 4-agent group load is one 8KB-contiguous run per partition.
        gT_c = np.ascontiguousarray(
            gT_c.transpose(1, 0, 2).reshape(DIN, A * BS))
        in_maps.append({"gT": gT_c, "w1": w1h, "w2": w2h,
                        "b1t": b1h, "b2t": b2h})
    return in_maps


def assemble_output(results, x, u, out_idx):
    """Gather per-core oT outputs, un-transpose, apply out_idx scatter."""
    o_rows = np.concatenate(
        [np.asarray(results[c]["outT"], dtype=np.float32)
         for c in range(N_CORES)], axis=1)                # [A*DOUT, B]
    o_flat = np.ascontiguousarray(o_rows.T)               # [B, 896]
    oi = np.asarray(out_idx).reshape(-1).astype(np.int64)
    if np.array_equal(oi, np.arange(A * DOUT)):
        return o_flat
    feats = np.concatenate([np.asarray(x, np.float32),
                            np.asarray(u, np.float32)], axis=1)
    feats[:, oi] = o_flat
    return np.ascontiguousarray(feats[:, :NX])


def kernel(x, u, W1, b1, W2, b2, in_idx, out_idx):
    nc = build_program(repeat=1)
    in_maps = prep_inputs(x, u, W1, b1, W2, b2, in_idx)
    res = run_bass_kernel_spmd(nc, in_maps, core_ids=list(range(N_CORES)))
    return assemble_output(res.results, x, u, out_idx)


# revision 18
# speedup vs baseline: 1.0871x; 1.0791x over previous
"""Trainium2 Bass kernel v7 for nn_GeneralNetworkedAE (gnn_message_passing).

Measured subsystem floors on current hw (ablate.py, repeat-delta):
  DMA  (fp8 gT + bf16 out, 12.1 MB/iter):        26.8 us
  PE   (256 mm1 + 256 mm2 matmuls + LDW):        55.7 us
  DVE evac 1244 ns / ACT evac 1038 ns per FD=1024 PSUM->SBUF instr
  pe_evac (full compute structure, no DMA):      70.8 us   <- the wall

The evacuation is the hard wall: PSUM reads are 1 elem/cycle/lane, f32
only on TRN2, and only DVE+ACT have PSUM ports.  Concurrent DMA adds a
further ~10-20% slowdown to the compute engines (pure resource
contention, not dependencies — measured via pe_evac_gx).  A/B tests
(interleaved, same hw state — absolute timings drift +-15% between
runs): out-DMAs on gpsimd/SWDGE beat sync/HWDGE; gT loads must stay on
sync (SWDGE loads: catastrophic); FD=2048 evacs are blocked by the
8-bank PSUM budget (4-bank tiles x2 bufs leave no room for mm2, and
hpsum bufs=2 + 2-bank mm2 tiles measured 1.49x WORSE).

Interleaved A/B results (drift-robust, med of per-round ratios):
  flat gT layout [DIN, A*BS] (8KB contiguous runs, 1024 descs/iter
  vs 4096):                               0.974x  -> ADOPTED (default)
  per-agent load granularity (g_split=4): 1.122x  -> rejected
  gpool_bufs 4 vs 3:                      wash    -> keep 3
  hpsum needs 3 bufs so DVE and ACT can drain two tiles concurrently
  while PE fills a third (2 bufs measured 1.49x worse).

Final config: gT in fp8 e3m4, flat [DIN, A*BS] layout (half DMA
traffic; end-to-end rel err ~1.5e-2 < 2e-2 with bf16 W1 — mixed-dtype
matmul), outT unpadded [896, BS] bf16 via gpsimd/SWDGE, mm2
software-pipelined one group behind mm1, DVE/ACT evacuation split by
measured per-instr costs.  Measured 89-96 us across hw states
(baseline v4: 92-94 us in the same windows).
"""

import numpy as np
import ml_dtypes

import concourse.bacc as bacc
import concourse.tile as tile
from concourse import mybir
from concourse.bass_utils import run_bass_kernel_spmd

BF16 = ml_dtypes.bfloat16
FP8 = ml_dtypes.float8_e3m4

B, NX, NU = 16384, 896, 128
A, DIN, H, DOUT = 32, 128, 256, 28
DOUTP = 32
N_CORES = 8
BS = B // N_CORES     # 2048 batch rows per core
BT = 512              # matmul moving free dim / psum bank
NT = BS // BT         # 4 batch tiles
NG = A // 4           # 8 groups of 4 agents

F32 = mybir.dt.float32
BF = mybir.dt.bfloat16
F8 = mybir.dt.float8e3


class EvacBalancer:
    def __init__(self):
        self.dve_ns = 0.0
        self.act_ns = 0.0

    def pick(self, fd):
        dve_cost = (120 + fd) / 0.96
        act_cost = (172 + fd) / 1.2
        if self.dve_ns + dve_cost <= self.act_ns + act_cost:
            self.dve_ns += dve_cost
            return "dve"
        self.act_ns += act_cost
        return "act"


def build_program(repeat: int = 1, out_eng: str = "gpsimd",
                  mm2_half: int = 0, gpool_bufs: int = 3,
                  hpsum_bufs: int = 3, flat_g: int = 1, g_split: int = 1,
                  out_pad: int = 0):
    nc = bacc.Bacc(trn_type="TRN2", target_bir_lowering=False, debug=False,
                   enable_asserts=True)
    if flat_g:
        gT = nc.dram_tensor("gT", [DIN, A * BS], F8, kind="ExternalInput").ap()
    else:
        gT = nc.dram_tensor("gT", [A, DIN, BS], F8, kind="ExternalInput").ap()
    w1 = nc.dram_tensor("w1", [DIN, A * H], BF, kind="ExternalInput").ap()
    w2 = nc.dram_tensor("w2", [128, A * 2 * DOUTP], BF, kind="ExternalInput").ap()
    b1t = nc.dram_tensor("b1t", [128, A * 2], F32, kind="ExternalInput").ap()
    b2t = nc.dram_tensor("b2t", [128, NG], F32, kind="ExternalInput").ap()
    out_rows = A * DOUTP if out_pad else A * DOUT
    outT = nc.dram_tensor("outT", [out_rows, BS], BF, kind="ExternalOutput").ap()

    add = mybir.AluOpType.add
    mx = mybir.AluOpType.max
    relu = mybir.ActivationFunctionType.Relu
    ident = mybir.ActivationFunctionType.Identity

    bal = EvacBalancer()

    out_dma = nc.gpsimd.dma_start if out_eng == "gpsimd" else nc.sync.dma_start
    with tile.TileContext(nc) as tc:
        with (
            tc.tile_pool(name="wpool", bufs=1) as wpool,
            tc.tile_pool(name="gpool", bufs=gpool_bufs) as gpool,
            tc.tile_pool(name="hpool", bufs=18) as hpool,
            tc.tile_pool(name="opool", bufs=2) as opool,
            tc.tile_pool(name="hpsum", bufs=(2 if mm2_half else hpsum_bufs),
                         space="PSUM") as hpsum,
            tc.tile_pool(name="opsum", bufs=2, space="PSUM") as opsum,
        ):
            w1_head = wpool.tile([DIN, 4 * H], BF)
            nc.sync.dma_start(out=w1_head[:], in_=w1[:, :4 * H])
            w1_tail = wpool.tile([DIN, (A - 4) * H], BF)
            nc.gpsimd.dma_start(out=w1_tail[:], in_=w1[:, 4 * H:])

            def w1_slice(a, m):
                if a < 4:
                    return w1_head[:, a * H + m * 128:a * H + (m + 1) * 128]
                b = a - 4
                return w1_tail[:, b * H + m * 128:b * H + (m + 1) * 128]
            w2_sb = wpool.tile([128, A * 2 * DOUTP], BF)
            nc.gpsimd.dma_start(out=w2_sb[:], in_=w2[:])
            b1_sb = wpool.tile([128, A * 2], F32)
            nc.gpsimd.dma_start(out=b1_sb[:], in_=b1t[:])
            b2_sb = wpool.tile([128, NG], F32)
            nc.gpsimd.dma_start(out=b2_sb[:], in_=b2t[:])

            def evac(out_ap, in_ap, bcol, do_relu, fd):
                if bal.pick(fd) == "dve":
                    if do_relu:
                        nc.vector.tensor_scalar(
                            out=out_ap, in0=in_ap,
                            scalar1=bcol, scalar2=0.0, op0=add, op1=mx)
                    else:
                        nc.vector.tensor_scalar(
                            out=out_ap, in0=in_ap,
                            scalar1=bcol, scalar2=None, op0=add)
                else:
                    nc.scalar.activation(
                        out=out_ap, in_=in_ap,
                        func=(relu if do_relu else ident),
                        bias=bcol, scale=1.0)

            def emit_mm2_chunk(p, t):
                """One batch-tile of mm2 for a finished group p."""
                pg = p["g"]
                if mm2_half:
                    if t % 2 == 0:
                        ps_o2 = opsum.tile([128, 2 * BT], F32, tag="po")
                        p["ps_o"] = ps_o2
                    ps_o = p["ps_o"][:, (t % 2) * BT:(t % 2 + 1) * BT]
                else:
                    ps_o1 = opsum.tile([128, BT], F32, tag="po")
                    ps_o = ps_o1[:, :]
                for m in range(2):
                    for j in range(4):
                        a = 4 * pg + j
                        nc.tensor.matmul(
                            ps_o[32 * j:32 * j + DOUTP, :],
                            lhsT=w2_sb[:, (a * 2 + m) * DOUTP:
                                       (a * 2 + m + 1) * DOUTP],
                            rhs=p["hts"][(j, m)][:, t * BT:(t + 1) * BT],
                            start=(m == 0), stop=(m == 1),
                            tile_position=(0, 32 * j),
                            skip_group_check=True,
                        )
                bcol = b2_sb[:, pg:pg + 1]
                if mm2_half:
                    if t % 2 == 1:
                        evac(p["ostage"][:, (t - 1) * BT:(t + 1) * BT],
                             p["ps_o"][:], bcol, False, 2 * BT)
                else:
                    evac(p["ostage"][:, t * BT:(t + 1) * BT],
                         ps_o, bcol, False, BT)
                if t == NT - 1:
                    if out_pad:
                        out_dma(out=outT[pg * 128:(pg + 1) * 128, :],
                                in_=p["ostage"][:])
                    else:
                        for j in range(4):
                            out_dma(
                                out=outT[pg * 4 * DOUT + j * DOUT:
                                         pg * 4 * DOUT + (j + 1) * DOUT, :],
                                in_=p["ostage"][32 * j:32 * j + DOUT, :])

            pending = None   # group whose mm2 lags one group behind
            for _r in range(repeat):
                for g in range(NG):
                    def g_src(a):
                        if flat_g:
                            return gT[:, a * BS:(a + 1) * BS]
                        return gT[a]

                    if g == 0 and _r == 0:
                        gts = []
                        for j in range(4):
                            g1 = wpool.tile([DIN, BS], F8, tag=f"g0a{j}")
                            nc.sync.dma_start(out=g1[:], in_=g_src(j))
                            gts.append(g1[:, :])
                    else:
                        gt4 = gpool.tile([DIN, 4 * BS], F8, tag="gt")
                        if g_split == 4:
                            for j in range(4):
                                nc.sync.dma_start(
                                    out=gt4[:, j * BS:(j + 1) * BS],
                                    in_=g_src(4 * g + j))
                        elif flat_g:
                            nc.sync.dma_start(
                                out=gt4[:],
                                in_=gT[:, 4 * g * BS:(4 * g + 4) * BS])
                        else:
                            nc.sync.dma_start(
                                out=gt4[:].rearrange("p (k c) -> p k c", k=4),
                                in_=gT[4 * g:4 * g + 4].rearrange(
                                    "k p c -> p k c"))
                        gts = [gt4[:, j * BS:(j + 1) * BS] for j in range(4)]
                    ostage = opool.tile([128, BS], BF, tag="ostage")

                    # ---- mm1 for group g, interleaved with mm2(g-1) ----
                    hts = {}
                    slot = 0
                    for j in range(4):
                        a = 4 * g + j
                        for m in range(2):
                            h_sb = hpool.tile([128, BS], BF, tag="h")
                            bcol = b1_sb[:, a * 2 + m:a * 2 + m + 1]
                            for half in range(2):          # t pairs
                                ps_h = hpsum.tile([128, 2 * BT], F32, tag="ph")
                                for tt in range(2):
                                    t = 2 * half + tt
                                    nc.tensor.matmul(
                                        ps_h[:, tt * BT:(tt + 1) * BT],
                                        lhsT=w1_slice(a, m),
                                        rhs=gts[j][:, t * BT:(t + 1) * BT],
                                        start=True, stop=True,
                                    )
                                evac(h_sb[:, half * 2 * BT:(half + 1) * 2 * BT],
                                     ps_h[:], bcol, True, 2 * BT)
                            hts[(j, m)] = h_sb
                            slot += 1
                            if pending is not None and slot % 2 == 0:
                                emit_mm2_chunk(pending, slot // 2 - 1)
                    pending = {"g": g, "hts": hts, "ostage": ostage}
            for t in range(NT):       # flush final group's mm2
                emit_mm2_chunk(pending, t)
            pending = None
    nc.compile()
    return nc


def prep_inputs(x, u, W1, b1, W2, b2, in_idx):
    """Host-side shard + layout prep. Returns per-core in_maps."""
    feats = np.concatenate([np.asarray(x, np.float32),
                            np.asarray(u, np.float32)], axis=1)  # [B, 1024]
    featsT = np.ascontiguousarray(feats.T).astype(FP8)           # [1024, B]
    flat_idx = np.asarray(in_idx).reshape(-1).astype(np.int64)
    gT_full = featsT[flat_idx]                                    # [A*DIN, B]

    w1h = np.asarray(W1, np.float32).transpose(1, 0, 2).reshape(DIN, A * H)
    w1h = np.ascontiguousarray(w1h).astype(BF16)
    w2p = np.zeros((A, H, DOUTP), np.float32)
    w2p[:, :, :DOUT] = np.asarray(W2, np.float32)
    w2h = (w2p.reshape(A, 2, 128, DOUTP).transpose(2, 0, 1, 3)
           .reshape(128, A * 2 * DOUTP))
    w2h = np.ascontiguousarray(w2h).astype(BF16)
    b1h = np.ascontiguousarray(
        np.asarray(b1, np.float32).reshape(A, 2, 128).transpose(2, 0, 1)
        .reshape(128, A * 2))
    b2h = np.zeros((128, NG), np.float32)
    for g in range(NG):
        for j in range(4):
            b2h[32 * j:32 * j + DOUT, g] = np.asarray(b2, np.float32)[4 * g + j]

    in_maps = []
    for c in range(N_CORES):
        gT_c = gT_full[:, c * BS:(c + 1) * BS].reshape(A, DIN, BS)
        # flat layout [DIN, A*BS]: agent-major within each partition row,
        # so a 4-agent group load is one 8KB-contiguous run per partition.
        gT_c = np.ascontiguousarray(
            gT_c.transpose(1, 0, 2).reshape(DIN, A * BS))
        in_maps.append({"gT": gT_c, "w1": w1h, "w2": w2h,
                        "b1t": b1h, "b2t": b2h})
    return in_maps


def assemble_output(results, x, u, out_idx):
    """Gather per-core oT outputs, un-transpose, apply out_idx scatter."""
    o_rows = np.concatenate(
        [np.asarray(results[c]["outT"], dtype=np.float32)
         for c in range(N_CORES)], axis=1)                # [A*DOUT, B]
    o_flat = np.ascontiguousarray(o_rows.T)               # [B, 896]
    oi = np.asarray(out_idx).reshape(-1).astype(np.int64)
    if np.array_equal(oi, np.arange(A * DOUT)):
        return o_flat
    feats = np.concatenate([np.asarray(x, np.float32),
                            np.asarray(u, np.float32)], axis=1)
    feats[:, oi] = o_flat
    return np.ascontiguousarray(feats[:, :NX])


def kernel(x, u, W1, b1, W2, b2, in_idx, out_idx):
    nc = build_program(repeat=1)
    in_maps = prep_inputs(x, u, W1, b1, W2, b2, in_idx)
    res = run_bass_kernel_spmd(nc, in_maps, core_ids=list(range(N_CORES)))
    return assemble_output(res.results, x, u, out_idx)
